# revision 1
# baseline (speedup 1.0000x reference)
"""Causal self-attention (B=2, T=2048, D=2048, H=16, hd=128, RoPE on masked
heads) as a Bass/Tile kernel on 8 Trainium2 NeuronCores.

Sharding: core c handles batch b=c//4 and heads 4*(c%4)..4*(c%4)+3 (data
parallel on B x tensor parallel on H).  Each core computes a partial output
projection y_b = O_local @ Wout_local^T; the host sums the 4 partials per
batch.

All heavy matmuls run as float32r (full-rate fp32 path on the PE array).
Layout strategy: host pre-transposes x and the weight slices so every device
matmul sees natural [contraction-on-partitions] operands; attention is
computed in transposed score space (S^T = K Q^T) so softmax normalization
becomes a per-free-element multiply and P^T feeds the PV matmul directly.
Scores are O(1) for this problem so softmax runs without max-subtraction;
the denominator comes from an all-ones matmul over P^T (replicated across
partitions so the normalizing multiply needs no broadcast step).
"""

import sys

sys.path.insert(0, "/opt/trn_rl_repo")

import numpy as np

import concourse.bass as bass
import concourse.mybir as mybir
import concourse.tile as tile
from concourse.bass_utils import run_bass_kernel_spmd

F32 = mybir.dt.float32
F32R = mybir.dt.float32r

B = 2
T = 2048
D = 2048
H = 16
HD = 128
N_CORES = 8
HEADS_PER_CORE = 4
CORES_PER_B = 4
P = 128
TB = 512          # t-block width for projections / attention q-tiles
KO = D // P       # 16 contraction subtiles for D-contraction
NTB = T // TB     # 4
NQK = 2 * HEADS_PER_CORE  # 8 q+k dout tiles of 128
SCALE = 1.0 / float(np.sqrt(HD))


# ---------------------------------------------------------------------------
# Walrus on this toolchain rejects instructions carrying more than one sync
# wait command; Tile can emit several (e.g. the kernel-tail drain).  Hoist
# the excess onto injected same-engine NoOps — semantically identical.
def _fix_waits(nc, cap=1):
    ctr = 0
    for f in nc.m.functions:
        for bb in f.blocks:
            insts = bb.instructions
            i = 0
            while i < len(insts):
                inst = insts[i]
                si = inst.sync_info
                if si is not None and si.on_wait and len(si.on_wait) > cap:
                    waits = list(si.on_wait)
                    keep, excess = waits[:cap], waits[cap:]
                    nops = []
                    for j in range(0, len(excess), cap):
                        ctr += 1
                        nops.append(
                            mybir.InstNoOp(
                                name=f"I-waitfix-{ctr}",
                                engine=inst.engine,
                                sync_info=mybir.SyncInfo(
                                    on_wait=excess[j : j + cap], on_update=[]
                                ),
                            )
                        )
                    inst.sync_info = mybir.SyncInfo(
                        on_wait=keep, on_update=list(si.on_update or [])
                    )
                    insts[i:i] = nops
                    i += len(nops)
                i += 1
    return ctr


def _phase1(nc, tc, xT, wqkT, wvT, qkT_scr, v_scr, qT0, kT0):
    with (
        tc.tile_pool(name="p1w", bufs=1) as p1w,
        tc.tile_pool(name="p1x", bufs=2) as p1x,
        tc.tile_pool(name="p1s", bufs=3) as p1s,
        tc.tile_pool(name="p1p", bufs=8, space="PSUM") as p1p,
    ):
        # Per-ko tiles + interleaved emission so the first accumulation
        # group starts as soon as its (weight, x) slice pair lands instead
        # of stalling on the whole 16 MB load.
        wqk_r = wqkT.rearrange("(ko p) d -> p ko d", p=P)
        wv_r = wvT.rearrange("(ko p) d -> p ko d", p=P)
        xT_r = xT.rearrange("(ko p) t -> p ko t", p=P)

        wqk_sb = []
        wv_sb = []
        xt0 = []
        for ko in range(KO):
            w = p1w.tile([P, NQK * P], F32R, tag=f"wqk{ko}", name=f"wqk{ko}")
            nc.sync.dma_start(w[:], wqk_r[:, ko])
            wqk_sb.append(w)
            x = p1x.tile([P, TB], F32R, tag=f"xt{ko}", name=f"xt0_{ko}")
            nc.sync.dma_start(x[:], xT_r[:, ko, 0:TB])
            xt0.append(x)
        for ko in range(KO):
            w = p1w.tile([P, HEADS_PER_CORE * HD], F32R, tag=f"wv{ko}", name=f"wv{ko}")
            nc.sync.dma_start(w[:], wv_r[:, ko])
            wv_sb.append(w)

        for tb in range(NTB):
            if tb == 0:
                xt = xt0
            else:
                xt = []
                for ko in range(KO):
                    x = p1x.tile([P, TB], F32R, tag=f"xt{ko}", name=f"xt{tb}_{ko}")
                    nc.sync.dma_start(x[:], xT_r[:, ko, tb * TB : (tb + 1) * TB])
                    xt.append(x)
            tsl = slice(tb * TB, (tb + 1) * TB)
            # Sweep 1: all 8 q,k dout groups, ko-outer / dout-inner — each
            # arriving (wqk, x) slice pair immediately feeds 8 matmuls so
            # the initial DMA fill overlaps compute. Sweep 2: the 4 v
            # groups (wv loads arrive during sweep 1).
            ps_qk = {d: p1p.tile([P, TB], F32, tag="ps1", name=f"ps_qk{tb}_{d}") for d in range(NQK)}
            for ko in range(KO):
                st, sp = (ko == 0), (ko == KO - 1)
                for d in range(NQK):
                    nc.tensor.matmul(
                        ps_qk[d][:],
                        wqk_sb[ko][:, d * P : (d + 1) * P],
                        xt[ko][:],
                        start=st,
                        stop=sp,
                    )
            for d in range(NQK):
                # head 0's q (d=0) and k (d=4) stay in SBUF — no DRAM
                # round-trip for the first attention head.  Copies alternate
                # DVE/ACT so the end-of-phase copy backlog halves.
                cp = (nc.vector.tensor_copy if (tb < NTB - 1 or d % 4 != 1)
                      else nc.scalar.copy)
                if d == 0:
                    cp(qT0[:, tsl], ps_qk[d][:])
                elif d == HEADS_PER_CORE:
                    cp(kT0[:, tsl], ps_qk[d][:])
                else:
                    sb = p1s.tile([P, TB], F32R, tag="sb1", name=f"sbq{tb}_{d}")
                    cp(sb[:], ps_qk[d][:])
                    nc.sync.dma_start(qkT_scr[d * P : (d + 1) * P, tsl], sb[:])
            ps_v = {t4: p1p.tile([P, HEADS_PER_CORE * HD], F32, tag="ps1", name=f"ps_v{tb}_{t4}") for t4 in range(4)}
            for ko in range(KO):
                st, sp = (ko == 0), (ko == KO - 1)
                for t4 in range(4):
                    nc.tensor.matmul(
                        ps_v[t4][:],
                        xt[ko][:, t4 * P : (t4 + 1) * P],
                        wv_sb[ko][:],
                        start=st,
                        stop=sp,
                    )
            for t4 in range(4):
                sb = p1s.tile([P, HEADS_PER_CORE * HD], F32R, tag="sb1", name=f"sbv{tb}_{t4}")
                (nc.vector.tensor_copy if (tb < NTB - 1 or t4 % 2 == 0)
                 else nc.scalar.copy)(sb[:], ps_v[t4][:])
                nc.sync.dma_start(
                    v_scr[tb * TB + t4 * P : tb * TB + (t4 + 1) * P, :], sb[:]
                )


def _phase2(nc, tc, outT, qkT_scr, v_scr, jT_sb, mask_sb, ones_sb, cs, qT0, kT0):
    with (
        tc.tile_pool(name="p2qk", bufs=2) as p2qk,
        tc.tile_pool(name="p2r", bufs=2) as p2r,
        tc.tile_pool(name="p2v", bufs=2) as p2v,
        tc.tile_pool(name="p2cs", bufs=2) as p2cs,
        tc.tile_pool(name="p2pt", bufs=8) as p2pt,
        tc.tile_pool(name="p2rec", bufs=4) as p2rec,
        tc.tile_pool(name="p2ps", bufs=4, space="PSUM") as p2ps,
        tc.tile_pool(name="p2po", bufs=2, space="PSUM") as p2po,
        tc.tile_pool(name="p2pd", bufs=2, space="PSUM") as p2pd,
    ):
        def st_tile(name):
            return p2ps.tile([P, TB], F32, tag="st", name=name)

        def load_head(h):
            if h == 0:
                qT_h, kT_h = qT0, kT0
            else:
                qT_h = p2qk.tile([P, T], F32R, tag="qT", name=f"qT{h}")
                kT_h = p2qk.tile([P, T], F32R, tag="kT", name=f"kT{h}")
                nc.sync.dma_start(qT_h[:], qkT_scr[h * P : (h + 1) * P, :])
                nc.sync.dma_start(
                    kT_h[:],
                    qkT_scr[(HEADS_PER_CORE + h) * P : (HEADS_PER_CORE + h + 1) * P, :],
                )
            cs_h = p2cs.tile([P, 2, T], F32, tag="cs", name=f"cs{h}")
            nc.sync.dma_start(cs_h[:], cs[h].rearrange("c p t -> p c t"))
            v_h = p2v.tile([P, T // P, HD], F32R, tag="vh", name=f"vh{h}")
            v_r = v_scr[:, h * HD : (h + 1) * HD].rearrange("(ko p) hd -> p ko hd", p=P)
            nc.sync.dma_start(v_h[:], v_r)
            return qT_h, kT_h, cs_h, v_h

        def rope_block(h, qr, kr, qT_h, kT_h, cs_h, tb):
            # RoPE for one 512-wide t-block: roped = C*q + S*(J q)
            sl = slice(tb * TB, (tb + 1) * TB)
            for src_t, dst in ((qT_h, qr), (kT_h, kr)):
                psj = st_tile(f"psj{h}{tb}")
                nc.tensor.matmul(psj[:], jT_sb[:], src_t[:, sl], start=True, stop=True)
                tmp = p2pt.tile([P, TB], F32, tag="ropetmp", name=f"tmp{h}{tb}")
                nc.vector.tensor_tensor(
                    tmp[:], psj[:], cs_h[:, 1, sl], mybir.AluOpType.mult
                )
                nc.vector.tensor_tensor(
                    dst[:, sl], src_t[:, sl], cs_h[:, 0, sl], mybir.AluOpType.mult
                )
                nc.vector.tensor_tensor(
                    dst[:, sl], dst[:, sl], tmp[:], mybir.AluOpType.add
                )

        def alloc_roped(h):
            qr = p2r.tile([P, T], F32R, tag="qr", name=f"qr{h}")
            kr = p2r.tile([P, T], F32R, tag="kr", name=f"kr{h}")
            return qr, kr

        def attn_tq(h, tq, qr, kr, v_h, pending):
            """Emit one q-tile of attention, software-pipelined: each ST is
            issued one block ahead of its PV/ones pair (carried in `pending`,
            a 1-deep list of (issue_pv_fn, pt))."""
            sl = slice(tq * TB, (tq + 1) * TB)
            nk = (tq + 1) * (TB // P)  # causal: only tk blocks up to diagonal
            ps_o = p2po.tile([P, TB], F32, tag="po", name=f"po{h}{tq}")
            ps_d = p2pd.tile([P, TB], F32, tag="pd", name=f"pd{h}{tq}")

            def issue_st(kb):
                ps_st = st_tile(f"st{h}{tq}{kb}")
                nc.tensor.matmul(
                    ps_st[:],
                    kr[:, kb * P : (kb + 1) * P],
                    qr[:, sl],
                    start=True,
                    stop=True,
                )
                pt = p2pt.tile([P, TB], F32R, tag="pt", name=f"pt{h}{tq}{kb}")
                nc.scalar.activation(
                    pt[:], ps_st[:], mybir.ActivationFunctionType.Exp, scale=SCALE
                )
                band = kb - tq * (TB // P)
                if band >= 0:
                    nc.vector.tensor_tensor(
                        pt[:], pt[:], mask_sb[:, band, :], mybir.AluOpType.mult
                    )
                return pt

            def make_pv(kb, pt):
                def pv():
                    nc.tensor.matmul(
                        ps_o[:], v_h[:, kb], pt[:], start=(kb == 0), stop=(kb == nk - 1)
                    )
                    nc.tensor.matmul(
                        ps_d[:], ones_sb[:], pt[:], start=(kb == 0), stop=(kb == nk - 1)
                    )
                    if kb == nk - 1:
                        rec = p2rec.tile([P, TB], F32, tag="rec", name=f"rec{h}{tq}")
                        nc.vector.reciprocal(rec[:], ps_d[:])
                        nc.vector.tensor_tensor(
                            outT[(h, tq)][:], ps_o[:], rec[:], mybir.AluOpType.mult
                        )
                return pv

            for kb in range(nk):
                pt = issue_st(kb)
                if len(pending) >= 4:
                    pending.pop(0)()
                pending.append(make_pv(kb, pt))

        # Loads run one head ahead; rope for head h+1 is interleaved into
        # head h's attention (one t-block per q-tile) so the DVE never has a
        # burst of blend work blocking the mask ops of the running head.
        loads = [load_head(0)]
        r0 = alloc_roped(0)
        for tb in range(NTB):
            rope_block(0, r0[0], r0[1], loads[0][0], loads[0][1], loads[0][2], tb)
        roped = [r0]
        pending = []
        for h in range(HEADS_PER_CORE):
            if h + 1 < HEADS_PER_CORE:
                loads.append(load_head(h + 1))
                roped.append(alloc_roped(h + 1))
            qr, kr = roped[h]
            for tq in range(NTB):
                attn_tq(h, tq, qr, kr, loads[h][3], pending)
                if h + 1 < HEADS_PER_CORE:
                    nh = loads[h + 1]
                    rope_block(h + 1, roped[h + 1][0], roped[h + 1][1],
                               nh[0], nh[1], nh[2], tq)
            if h == HEADS_PER_CORE - 1:
                while pending:
                    pending.pop(0)()


def _phase3(nc, tc, outT, woT, y):
    with (
        tc.tile_pool(name="p3w", bufs=1) as p3w,
        tc.tile_pool(name="p3s", bufs=6) as p3s,
        tc.tile_pool(name="p3p", bufs=6, space="PSUM") as p3p,
    ):

        wo_sb = p3w.tile([P, HEADS_PER_CORE, D], F32R)
        nc.sync.dma_start(wo_sb[:], woT.rearrange("(h p) d -> p h d", p=P))
        for tq in range(NTB):
            for tt in range(tq * (TB // P), (tq + 1) * (TB // P)):
                off = (tt - tq * (TB // P)) * P
                for dd in range(D // TB):
                    ps = p3p.tile([P, TB], F32, tag="ps3", name=f"ps3{tt}{dd}")
                    for h in range(HEADS_PER_CORE):
                        nc.tensor.matmul(
                            ps[:],
                            outT[(h, tq)][:, off : off + P],
                            wo_sb[:, h, dd * TB : (dd + 1) * TB],
                            start=(h == 0),
                            stop=(h == HEADS_PER_CORE - 1),
                        )
                    sb = p3s.tile([P, TB], F32, tag="sb3", name=f"sb3{tt}{dd}")
                    (nc.vector.tensor_copy if dd % 2 == 0 else nc.scalar.copy)(sb[:], ps[:])
                    nc.sync.dma_start(
                        y[tt * P : (tt + 1) * P, dd * TB : (dd + 1) * TB], sb[:]
                    )


def _build_program():
    nc = bass.Bass()

    xT = nc.dram_tensor("xT", (D, T), F32R, kind="ExternalInput")
    wqkT = nc.dram_tensor("wqkT", (D, NQK * P), F32R, kind="ExternalInput")
    wvT = nc.dram_tensor("wvT", (D, HEADS_PER_CORE * HD), F32R, kind="ExternalInput")
    woT = nc.dram_tensor("woT", (HEADS_PER_CORE * HD, D), F32R, kind="ExternalInput")
    jT = nc.dram_tensor("jT", (P, P), F32R, kind="ExternalInput")
    ones = nc.dram_tensor("ones", (P, P), F32R, kind="ExternalInput")
    cs = nc.dram_tensor("cs", (HEADS_PER_CORE, 2, P, T), F32, kind="ExternalInput")
    masks = nc.dram_tensor("masks", (TB // P, P, TB), mybir.dt.bfloat16, kind="ExternalInput")
    y = nc.dram_tensor("y", (T, D), F32, kind="ExternalOutput")

    with tile.TileContext(nc) as tc:
        with (
            tc.tile_pool(name="dram", bufs=1, space="DRAM") as dram,
            tc.tile_pool(name="consts", bufs=1) as consts,
        ):
            qkT_scr = dram.tile([NQK * P, T], F32R)  # q rows then k rows
            v_scr = dram.tile([T, HEADS_PER_CORE * HD], F32R)

            jT_sb = consts.tile([P, P], F32R)
            nc.sync.dma_start(jT_sb[:], jT[:])
            mask_sb = consts.tile([P, TB // P, TB], mybir.dt.bfloat16)
            nc.sync.dma_start(mask_sb[:], masks.rearrange("a p j -> p a j"))
            ones_sb = consts.tile([P, P], F32R)
            nc.sync.dma_start(ones_sb[:], ones[:])

            qT0 = consts.tile([P, T], F32R)
            kT0 = consts.tile([P, T], F32R)
            _phase1(nc, tc, xT, wqkT, wvT, qkT_scr, v_scr, qT0, kT0)

            with tc.tile_pool(name="outT", bufs=1) as outT_pool:
                outT = {
                    (h, tq): outT_pool.tile(
                        [P, TB], F32R, tag=f"outT{h}_{tq}", name=f"outT{h}_{tq}"
                    )
                    for h in range(HEADS_PER_CORE)
                    for tq in range(NTB)
                }
                _phase2(nc, tc, outT, qkT_scr, v_scr, jT_sb, mask_sb, ones_sb, cs, qT0, kT0)
                _phase3(nc, tc, outT, woT, y)

    _fix_waits(nc)
    return nc


_NC_CACHE = None


def _get_program():
    global _NC_CACHE
    if _NC_CACHE is None:
        _NC_CACHE = _build_program()
    return _NC_CACHE


def _host_inputs(x, Wqkv, Wout, cos, sin, rope_mask):
    """Build the 8 per-core input maps."""
    x = np.asarray(x, dtype=np.float32)
    Wqkv = np.asarray(Wqkv, dtype=np.float32)
    Wout = np.asarray(Wout, dtype=np.float32)
    cos = np.asarray(cos, dtype=np.float32)
    sin = np.asarray(sin, dtype=np.float32)
    rope_mask = np.asarray(rope_mask).astype(bool)

    # J^T for the pair-rotation matmul: (J q)[2i] = -q[2i+1], (J q)[2i+1] = q[2i]
    jT = np.zeros((P, P), dtype=np.float32)
    for i in range(P // 2):
        jT[2 * i, 2 * i + 1] = 1.0
        jT[2 * i + 1, 2 * i] = -1.0

    # causal 0/1 masks for the diagonal band blocks: valid iff i + a*128 <= j
    import ml_dtypes
    masks = np.zeros((TB // P, P, TB), dtype=ml_dtypes.bfloat16)
    ii = np.arange(P)[:, None]
    jj = np.arange(TB)[None, :]
    for a in range(TB // P):
        masks[a] = (ii + a * P <= jj).astype(ml_dtypes.bfloat16)

    C_full = np.repeat(cos[:T].T, 2, axis=0).astype(np.float32)  # [128, T]
    S_full = np.repeat(sin[:T].T, 2, axis=0).astype(np.float32)
    C_id = np.ones_like(C_full)
    S_id = np.zeros_like(S_full)

    in_maps = []
    for c in range(N_CORES):
        b = c // CORES_PER_B
        hg = c % CORES_PER_B
        heads = [hg * HEADS_PER_CORE + i for i in range(HEADS_PER_CORE)]

        qrows = np.concatenate([np.arange(h * HD, (h + 1) * HD) for h in heads])
        krows = qrows + D
        vrows = qrows + 2 * D
        wqkT_l = np.ascontiguousarray(Wqkv[np.concatenate([qrows, krows])].T)
        wvT_l = np.ascontiguousarray(Wqkv[vrows].T)
        woT_l = np.ascontiguousarray(Wout[:, qrows].T)

        cs_arr = np.empty((HEADS_PER_CORE, 2, P, T), dtype=np.float32)
        for i, h in enumerate(heads):
            cs_arr[i, 0] = C_full if rope_mask[h] else C_id
            cs_arr[i, 1] = S_full if rope_mask[h] else S_id

        in_maps.append(
            {
                "xT": np.ascontiguousarray(x[b].T),
                "wqkT": wqkT_l,
                "wvT": wvT_l,
                "woT": woT_l,
                "jT": jT,
                "ones": np.ones((P, P), dtype=np.float32),
                "cs": cs_arr,
                "masks": masks,
            }
        )
    return in_maps


def kernel(x, Wqkv, Wout, cos, sin, rope_mask, _trace=False):
    nc = _get_program()
    in_maps = _host_inputs(x, Wqkv, Wout, cos, sin, rope_mask)
    res = run_bass_kernel_spmd(nc, in_maps, core_ids=list(range(N_CORES)), trace=_trace)
    parts = [res.results[c]["y"] for c in range(N_CORES)]
    out = np.stack(
        [sum(parts[b * CORES_PER_B : (b + 1) * CORES_PER_B]) for b in range(B)]
    ).astype(np.float32)
    if _trace:
        kernel.last_result = res
    return out

